# revision 1
# baseline (speedup 1.0000x reference)
"""GNN mean-aggregation (PyG MessagePassing, aggr='mean') on 8 TRN2 NeuronCores.

Sharding strategy (host): edges are partitioned by destination across the 8
cores (core c owns dst in [12500c, 12500(c+1))), and within each core further
partitioned into 98 sub-shards by 128-node destination block. Edges keep
their original relative order inside each sub-shard; sub-shards are padded to
a uniform chunk count so one compiled program serves every round/direction.

Device program "aggregate" (per core, per round):
  - indirect-DMA gather of x[src] rows (128 rows / instruction)
  - one-hot build for dst-lo via DVE is_equal against an iota tile
  - PE matmul accumulates one-hot^T @ msg into a PSUM-resident accumulator
    [128 lo x (98 blocks x 16 dims)]
  - counts come from running the same program with x = ones
Device program "divide": out = sums * reciprocal(max(cnt, 1)).
Host only reassembles the 8 per-core output slices into the full h between
rounds (allgather equivalent).
"""
import sys
sys.path.insert(0, '/opt/trn_rl_repo')
import numpy as np

import concourse.bass as bass
import concourse.tile as tile
from concourse import bacc, mybir
from concourse.bass_utils import run_bass_kernel_spmd

N_NODES = 100000
DIM = 16
N_EDGES = 3200000
N_CORES = 8
NLOC = N_NODES // N_CORES        # 12500 nodes per core
NBLK = (NLOC + 127) // 128       # 98 blocks per core
PAD_LO = 128                     # one-hot sentinel (never matches iota 0..127)

_PROGRAMS = {}


def _shard(edge_index):
    """Partition edges by (core, dst-block); pad sub-shards to uniform U."""
    src = np.asarray(edge_index[0], dtype=np.int64)
    dst = np.asarray(edge_index[1], dtype=np.int64)
    core = dst // NLOC
    loc = dst - core * NLOC
    blk = loc // 128
    lo = loc % 128
    key = core * NBLK + blk
    order = np.argsort(key, kind='stable')
    ks, ss, ls = key[order], src[order], lo[order]
    counts = np.bincount(ks, minlength=N_CORES * NBLK).reshape(N_CORES, NBLK)
    U = int((counts.max() + 127) // 128)
    gsrc = np.zeros((N_CORES, NBLK * U * 128), np.int32)
    glo = np.full((N_CORES, NBLK * U * 128), PAD_LO, np.float32)
    starts = np.zeros(N_CORES * NBLK + 1, np.int64)
    np.cumsum(counts.ravel(), out=starts[1:])
    for c in range(N_CORES):
        for b in range(NBLK):
            k = c * NBLK + b
            n = counts[c, b]
            s0 = starts[k]
            base = (b * U) * 128
            gsrc[c, base:base + n] = ss[s0:s0 + n]
            glo[c, base:base + n] = ls[s0:s0 + n]
    return gsrc, glo, U


def _build_aggregate(U):
    NS = NBLK * U                      # chunk slots per core
    nc = bacc.Bacc("TRN2", target_bir_lowering=False, debug=False,
                   num_devices=N_CORES)
    h_in = nc.dram_tensor("h", [N_NODES, DIM], mybir.dt.float32,
                          kind="ExternalInput")
    gsrc = nc.dram_tensor("gsrc", [128, NS], mybir.dt.int32,
                          kind="ExternalInput")   # slot-major, wrapped to 128 partitions
    iotaf = nc.dram_tensor("iotaf", [128, 128], mybir.dt.float32,
                           kind="ExternalInput")
    glo = nc.dram_tensor("glo", [128, NS], mybir.dt.float32,
                         kind="ExternalInput")
    sums = nc.dram_tensor("sums", [128, NBLK * DIM], mybir.dt.float32,
                          kind="ExternalOutput")
    with tile.TileContext(nc) as tc:
        with (
            tc.tile_pool(name="const", bufs=1) as constp,
            tc.tile_pool(name="idx", bufs=1) as idxp,
            tc.tile_pool(name="msg", bufs=24) as msgp,
            tc.tile_pool(name="oh", bufs=24) as ohp,
            tc.tile_pool(name="accs", bufs=1) as accp,
            tc.tile_pool(name="psum", bufs=1, space="PSUM") as psump,
        ):
            iota = constp.tile([128, 128], mybir.dt.float32)
            nc.sync.dma_start(out=iota[:], in_=iotaf.ap()[:])
            idx_t = idxp.tile([128, NS], mybir.dt.int32)
            nc.sync.dma_start(out=idx_t[:], in_=gsrc.ap()[:])
            lo_t = idxp.tile([128, NS], mybir.dt.float32)
            nc.sync.dma_start(out=lo_t[:], in_=glo.ap()[:])
            acc = psump.tile([128, NBLK * DIM], mybir.dt.float32, space="PSUM")
            for b in range(NBLK):
                for u in range(U):
                    s = b * U + u
                    msg = msgp.tile([128, DIM], mybir.dt.float32, tag="msg")
                    nc.gpsimd.indirect_dma_start(
                        out=msg[:], out_offset=None, in_=h_in.ap()[:],
                        in_offset=bass.IndirectOffsetOnAxis(
                            ap=idx_t[:, s:s + 1], axis=0))
                    oh = ohp.tile([128, 128], mybir.dt.float32, tag="oh")
                    nc.vector.tensor_tensor(
                        out=oh[:], in0=lo_t[:, s:s + 1].to_broadcast([128, 128]),
                        in1=iota[:], op=mybir.AluOpType.is_equal)
                    nc.tensor.matmul(
                        out=acc[:, b * DIM:(b + 1) * DIM], lhsT=oh[:], rhs=msg[:],
                        start=(u == 0), stop=(u == U - 1))
            accs = accp.tile([128, NBLK * DIM], mybir.dt.float32)
            nc.vector.tensor_copy(out=accs[:], in_=acc[:])
            nc.sync.dma_start(out=sums.ap()[:], in_=accs[:])
    nc.compile()
    return nc


def _build_divide():
    nc = bacc.Bacc("TRN2", target_bir_lowering=False, debug=False,
                   num_devices=N_CORES)
    s_in = nc.dram_tensor("s", [128, NBLK * DIM], mybir.dt.float32, kind="ExternalInput")
    c_in = nc.dram_tensor("c", [128, NBLK * DIM], mybir.dt.float32, kind="ExternalInput")
    h_out = nc.dram_tensor("o", [128, NBLK * DIM], mybir.dt.float32, kind="ExternalOutput")
    with tile.TileContext(nc) as tc:
        with tc.tile_pool(name="p", bufs=2) as pool:
            st = pool.tile([128, NBLK * DIM], mybir.dt.float32, tag="s")
            nc.sync.dma_start(out=st[:], in_=s_in.ap()[:])
            ct = pool.tile([128, NBLK * DIM], mybir.dt.float32, tag="c")
            nc.sync.dma_start(out=ct[:], in_=c_in.ap()[:])
            cm = pool.tile([128, NBLK * DIM], mybir.dt.float32, tag="cm")
            nc.vector.tensor_scalar_max(out=cm[:], in0=ct[:], scalar1=1.0)
            cr = pool.tile([128, NBLK * DIM], mybir.dt.float32, tag="cr")
            nc.vector.reciprocal(out=cr[:], in_=cm[:])
            ot = pool.tile([128, NBLK * DIM], mybir.dt.float32, tag="o")
            nc.vector.tensor_mul(out=ot[:], in0=st[:], in1=cr[:])
            nc.sync.dma_start(out=h_out.ap()[:], in_=ot[:])
    nc.compile()
    return nc


def _wrap_slots(arr):
    # [NS*128] slot-major -> [128, NS] partition-wrapped (edge e of slot s at
    # partition e, column s)
    ns = arr.shape[-1] // 128
    return np.ascontiguousarray(arr.reshape(ns, 128).T)


def _run_aggregate(prog, h_full, gsrc_w, glo_w):
    core_ids = list(range(N_CORES))
    iota_np = np.tile(np.arange(128, dtype=np.float32), (128, 1))
    in_maps = [{"h": h_full, "gsrc": gsrc_w[c], "glo": glo_w[c], "iotaf": iota_np}
               for c in range(N_CORES)]
    res = run_bass_kernel_spmd(prog, in_maps, core_ids)
    return [res.results[c]["sums"] for c in range(N_CORES)]


def _run_divide(prog, sums_list, cnts_list):
    core_ids = list(range(N_CORES))
    in_maps = [{"s": sums_list[c], "c": cnts_list[c]} for c in range(N_CORES)]
    res = run_bass_kernel_spmd(prog, in_maps, core_ids)
    h = np.empty((N_NODES, DIM), np.float32)
    for c in range(N_CORES):
        o = res.results[c]["o"].reshape(128, NBLK, DIM).transpose(1, 0, 2)
        h[c * NLOC:(c + 1) * NLOC] = o.reshape(NBLK * 128, DIM)[:NLOC]
    return h


def kernel(topic_entity_one_hot, edge_index, reverse_edge_index):
    x = np.asarray(topic_entity_one_hot, dtype=np.float32)
    shards = [_shard(np.asarray(edge_index)),
              _shard(np.asarray(reverse_edge_index))]
    U = max(s[2] for s in shards)
    # re-shard with the common U so both directions fit one program
    def repad(ei):
        gsrc, glo, _ = _shard_fixed(np.asarray(ei), U)
        return gsrc, glo
    fwd = repad(edge_index)
    rev = repad(reverse_edge_index)

    if ("agg", U) not in _PROGRAMS:
        _PROGRAMS[("agg", U)] = _build_aggregate(U)
    if "div" not in _PROGRAMS:
        _PROGRAMS["div"] = _build_divide()
    agg, div = _PROGRAMS[("agg", U)], _PROGRAMS["div"]

    results = []
    ones = np.ones((N_NODES, DIM), np.float32)
    for (gsrc, glo) in (fwd, rev):
        gsrc_w = [_wrap_slots(gsrc[c]) for c in range(N_CORES)]
        glo_w = [_wrap_slots(glo[c]) for c in range(N_CORES)]
        cnts = _run_aggregate(agg, ones, gsrc_w, glo_w)
        h = x
        for _ in range(2):
            sums = _run_aggregate(agg, h, gsrc_w, glo_w)
            h = _run_divide(div, sums, cnts)
            results.append(h)
    out = np.stack([results[0], results[1], results[2], results[3]], axis=0)
    return out


def _shard_fixed(edge_index, U):
    src = np.asarray(edge_index[0], dtype=np.int64)
    dst = np.asarray(edge_index[1], dtype=np.int64)
    core = dst // NLOC
    loc = dst - core * NLOC
    blk = loc // 128
    lo = loc % 128
    key = core * NBLK + blk
    order = np.argsort(key, kind='stable')
    ks, ss, ls = key[order], src[order], lo[order]
    counts = np.bincount(ks, minlength=N_CORES * NBLK).reshape(N_CORES, NBLK)
    assert counts.max() <= U * 128
    gsrc = np.zeros((N_CORES, NBLK * U * 128), np.int32)
    glo = np.full((N_CORES, NBLK * U * 128), PAD_LO, np.float32)
    starts = np.zeros(N_CORES * NBLK + 1, np.int64)
    np.cumsum(counts.ravel(), out=starts[1:])
    for c in range(N_CORES):
        for b in range(NBLK):
            k = c * NBLK + b
            n = counts[c, b]
            s0 = starts[k]
            base = (b * U) * 128
            gsrc[c, base:base + n] = ss[s0:s0 + n]
            glo[c, base:base + n] = ls[s0:s0 + n]
    return gsrc, glo, U

